# revision 48
# baseline (speedup 1.0000x reference)
"""BiasAttention TRN2 kernel — q-sharded across 8 NeuronCores.

Each core owns a block of 128 queries and computes the full attention for
them (all 8 heads, all 1024 keys), including the z-bias projection, with no
collectives.  The whole attention is computed TRANSPOSED (S^T[k,q],
bias^T[k,q]) so the exp'd scores feed the PV matmul directly with no PE
transposes.  Host-side prep lays z out per core as [g, c, tq, k] (contract
dim c on partitions, one query's [c,k] tile per stationary) and casts z to
fp8e4m3 — halving the dominant HBM stream; Wb stays bf16 so the
quantization error is z-only.  Exact-math folds: bb and the K-side bkv
bias are constant along the softmax axis (cancel), and the V-side bkv bias
folds into bp on host (softmax rows sum to 1).
"""

import sys

if "/opt/trn_rl_repo" not in sys.path:
    sys.path.insert(0, "/opt/trn_rl_repo")

import ml_dtypes
import numpy as np

import concourse.bass as bass
import concourse.mybir as mybir
from concourse import bacc
from concourse.bass_utils import run_bass_kernel_spmd
from concourse.masks import make_identity
from concourse.tile import TileContext

P = 128          # partitions
H = 8            # heads
D = 32           # head dim
CQ = 256         # q channels
CKV = 256        # kv channels
BD = 128         # bias (z) channels
NQ = 1024        # total queries
NCORES = 8
NQC = NQ // NCORES   # queries per core = 128
SCALE = D ** (-0.5)

TQ = 32          # queries per z DMA group (tile = [c, TQ, 128k], 4KB/part)
FP = mybir.dt.float32
BF = mybir.dt.bfloat16
F8 = mybir.dt.float8e4
NP_BF = ml_dtypes.bfloat16
NP_F8 = ml_dtypes.float8_e4m3


def build_program(nk=1024):
    kc_n = nk // P            # k-chunks of 128
    gpc = NQC // TQ           # z groups per k-chunk (4)
    ng = kc_n * gpc           # z DMA groups (32)
    add = mybir.AluOpType.add
    mult = mybir.AluOpType.mult

    nc = bacc.Bacc("TRN2", target_bir_lowering=False, debug=False,
                   num_devices=NCORES)

    # ---- I/O ----
    zT = nc.dram_tensor("zT", [ng, BD, TQ, P], F8, kind="ExternalInput")
    xqT = nc.dram_tensor("xqT", [CQ, NQC], BF, kind="ExternalInput")
    xkvT = nc.dram_tensor("xkvT", [CKV, nk], BF, kind="ExternalInput")
    Wq = nc.dram_tensor("Wq", [CQ, H * D], BF, kind="ExternalInput")
    bq = nc.dram_tensor("bq", [H * D], FP, kind="ExternalInput")
    Wkv = nc.dram_tensor("Wkv", [CKV, 2 * H * D], BF, kind="ExternalInput")
    Wb = nc.dram_tensor("Wb", [BD, H], BF, kind="ExternalInput")
    Wp = nc.dram_tensor("Wp", [H * D, CQ], FP, kind="ExternalInput")
    bp = nc.dram_tensor("bp", [CQ], FP, kind="ExternalInput")
    y = nc.dram_tensor("y", [NQC, CQ], FP, kind="ExternalOutput")

    with TileContext(nc) as tc:
        with (
            tc.tile_pool(name="const", bufs=1) as const,
            tc.tile_pool(name="zpool", bufs=26) as zpool,
            tc.tile_pool(name="epool", bufs=6) as epool,
            tc.tile_pool(name="xpool", bufs=6) as xpool,
            tc.tile_pool(name="proj_ps", bufs=2, space="PSUM") as proj_ps,
            tc.tile_pool(name="stv_ps", bufs=2, space="PSUM") as stv_ps,
            tc.tile_pool(name="b_ps", bufs=3, space="PSUM") as b_psp,
            tc.tile_pool(name="o_ps", bufs=1, space="PSUM") as o_psp,
        ):
            # ---- z stream leads the sync queue; a stalled z trigger only
            # head-of-line blocks other z triggers (and the final y store).
            wb_sb = const.tile([P, H], BF)
            nc.sync.dma_start(wb_sb, Wb[:])
            zs = {}
            for g in range(6):
                z_sb = zpool.tile([P, TQ, P], F8, tag="z", name=f"z{g}")
                nc.sync.dma_start(z_sb, zT[g])
                zs[g] = z_sb

            def z_fetch(g):
                if g not in zs:
                    z_sb = zpool.tile([P, TQ, P], F8, tag="z")
                    nc.sync.dma_start(z_sb, zT[g])
                    zs[g] = z_sb
                return zs[g]

            # ---- consts: big ones on the scalar HWDGE queue, tiny on
            # gpsimd software DGE.
            # K-side consts first: they gate the longest prologue chain
            # (K-proj -> kT32 shuffle -> S^T -> first adds).
            wkv_sb = const.tile([P, 2, 2 * H * D], BF)
            nc.scalar.dma_start(wkv_sb, Wkv.rearrange("(o p) m -> p o m", p=P))
            xkvT_sb = const.tile([P, 2, nk], BF)
            nc.scalar.dma_start(xkvT_sb, xkvT.rearrange("(o p) k -> p o k", p=P))
            wq_sb = const.tile([P, 2, H * D], BF)
            nc.scalar.dma_start(wq_sb, Wq.rearrange("(o p) m -> p o m", p=P))
            xqT_sb = const.tile([P, 2, NQC], BF)
            nc.scalar.dma_start(xqT_sb, xqT.rearrange("(o p) q -> p o q", p=P))
            wp_sb = const.tile([P, 2, CQ], FP)
            nc.scalar.dma_start(wp_sb, Wp.rearrange("(o p) m -> p o m", p=P))
            bq_sb = const.tile([P, 2], FP)
            nc.gpsimd.dma_start(bq_sb, bq.rearrange("(o p) -> p o", p=P))
            bp_sb = const.tile([1, CQ], FP)
            nc.gpsimd.dma_start(bp_sb, bp[None, :])
            ident = const.tile([P, P], FP)
            make_identity(nc, ident)
            ones_row = const.tile([1, P], FP)
            nc.vector.memset(ones_row, 1.0)

            # V augmented with a ones column per head: [k, kc, h, D+1]
            vaug_sb = const.tile([P, kc_n, H, D + 1], BF)
            nc.vector.memset(vaug_sb, 1.0)

            # ---- projections (bf16 in, fp32 psum accumulate) ----
            # Q^T [(h d), q] with (x + bq) * SCALE folded in, stored bf16
            qT_sb = const.tile([P, 2, NQC], BF)
            for m in range(2):
                ps = proj_ps.tile([P, 512], FP, tag="proj")
                for c in range(2):
                    nc.tensor.matmul(ps[:, :NQC],
                                     lhsT=wq_sb[:, c, m * P:(m + 1) * P],
                                     rhs=xqT_sb[:, c, :],
                                     start=(c == 0), stop=(c == 1))
                nc.vector.tensor_scalar(qT_sb[:, m, :], ps[:, :NQC],
                                        bq_sb[:, m:m + 1], SCALE, add, mult)

            # K^T [(h d), k] — bkv_K is softmax-invariant, dropped
            kT_sb = const.tile([P, 2, nk], BF)
            for m in range(2):
                for nh in range((nk + 511) // 512):
                    nn_ = min(512, nk - nh * 512)
                    ps = proj_ps.tile([P, 512], FP, tag="proj")
                    for c in range(2):
                        nc.tensor.matmul(ps[:, :nn_],
                                         lhsT=wkv_sb[:, c, m * P:(m + 1) * P],
                                         rhs=xkvT_sb[:, c, nh * 512:nh * 512 + nn_],
                                         start=(c == 0), stop=(c == 1))
                    if (m * 2 + nh) % 2 == 0:
                        nc.scalar.activation(
                            kT_sb[:, m, nh * 512:nh * 512 + nn_], ps[:, :nn_],
                            mybir.ActivationFunctionType.Copy)
                    else:
                        nc.vector.tensor_copy(
                            kT_sb[:, m, nh * 512:nh * 512 + nn_], ps[:, :nn_])

            # qT/kT reshuffled so every head's d-dim sits on partitions
            # 0-31: the S^T matmuls then run at default tile position and
            # can share psum banks at different column offsets (the same
            # pattern the z matmuls use) — tile-positioned matmuls sharing
            # a bank wedge the device.  One SBUF->SBUF DMA per head.
            qT32 = const.tile([32, H, NQC], BF)
            kT32 = const.tile([32, H, nk], BF)
            for hq in range(4):
                nc.gpsimd.dma_start(qT32[:, hq::4, :],
                                    qT_sb[hq * 32:(hq + 1) * 32, :, :])
            for m in range(2):
                for hq in range(4):
                    nc.scalar.dma_start(kT32[:, m * 4 + hq, :],
                                        kT_sb[hq * 32:(hq + 1) * 32, m, :])

            # S^T[k, kc, h, q]: per (kc, head) a [k=128, q=128] matmul;
            # 4 heads batch into one psum tile with a single copy out.
            sT_sb = const.tile([P, kc_n, H, NQC], FP)

            def st_build(kc):
                for hg in range(2):
                    ps = stv_ps.tile([P, 512], FP, tag="stv", name="st_ps")
                    for hl in range(4):
                        h = hg * 4 + hl
                        nc.tensor.matmul(ps[:, hl * P:(hl + 1) * P],
                                         lhsT=kT32[:, h, kc * P:(kc + 1) * P],
                                         rhs=qT32[:, h, :],
                                         start=True, stop=True)
                    nc.vector.tensor_copy(
                        sT_sb[:, kc, hg * 4:(hg + 1) * 4, :],
                        ps.rearrange("p (hl q) -> p hl q", hl=4))

            # V [k, (h d)] (bias folded into bp on host) into vaug
            def v_build(kc):
                ps = stv_ps.tile([P, 512], FP, tag="stv", name="v_ps")
                for c in range(2):
                    nc.tensor.matmul(ps[:, :H * D],
                                     lhsT=xkvT_sb[:, c, kc * P:(kc + 1) * P],
                                     rhs=wkv_sb[:, c, H * D:2 * H * D],
                                     start=(c == 0), stop=(c == 1))
                nc.vector.tensor_copy(
                    vaug_sb[:, kc, :, 0:D],
                    ps[:, :H * D].rearrange("p (h d) -> p h d", h=H))

            st_build(0)
            st_build(1)
            v_build(0)

            # ---- main loop over k-chunks, software-pipelined: PV for chunk
            # kc is emitted after the z matmuls of chunk kc+1, so by the
            # time the PE reaches it the exp has long finished (no PE stall
            # on the cross-engine add->exp chain).
            o_ps = o_psp.tile([P, H * (D + 1)], FP)   # [q, h*(D+1)]
            xTs = {}

            def pv_emit(kc):
                xT_sb = xTs.pop(kc)
                for h in range(H):
                    # o_ps lives in one bank: open the accumulation group
                    # on the first matmul only, close on the last.
                    nc.tensor.matmul(
                        o_ps[:, h * (D + 1):(h + 1) * (D + 1)],
                        lhsT=xT_sb[:, h, :], rhs=vaug_sb[:, kc, h, :],
                        start=(kc == 0 and h == 0),
                        stop=(kc == kc_n - 1 and h == H - 1))

            for kc in range(kc_n):
                # e^T holds the chunk's logits [k, h, q] (bf16)
                e_sb = epool.tile([P, H, NQC], BF, tag="e")
                bps = []
                for hf in range(2):
                    # bias^T: one [k, 8] matmul per query into b_ps
                    # [k, (q64 h8)]; 64 queries per psum bank.
                    b_ps = b_psp.tile([P, 64 * H], FP, tag="b")
                    bps.append(b_ps)
                    for qb in (2 * hf, 2 * hf + 1):
                        z_sb = z_fetch(kc * gpc + qb)
                        for t in range(TQ):
                            ql = (qb % 2) * TQ + t
                            nc.tensor.matmul(b_ps[:, ql * H:(ql + 1) * H],
                                             lhsT=z_sb[:, t, :], rhs=wb_sb,
                                             start=(ql == 0),
                                             stop=(ql == 63))
                # adds first (critical chain), then PV two chunks back (its
                # exp is long done), then the V / S^T prefetch builds.
                for hf in range(2):
                    bv = bps[hf].rearrange("p (q h) -> p h q", h=H)
                    qs = slice(hf * 64, (hf + 1) * 64)
                    for hg in range(2):
                        hh = slice(hg * 4, (hg + 1) * 4)
                        nc.vector.tensor_tensor(
                            e_sb[:, hh, qs], sT_sb[:, kc, hh, qs],
                            bv[:, hh, :], add)
                if kc - 2 in xTs:
                    pv_emit(kc - 2)
                if kc == kc_n - 1:
                    # drain the PV backlog: PV(kc-1)'s exp completed during
                    # this chunk's z matmuls, so it is stall-free here
                    pv_emit(kc - 1)
                # exps BEFORE the v/st builds: their psum copies queue
                # behind the exps on ACT, and the next chunk's v/st matmuls
                # wait on those copies through the stv ring — emitting exps
                # first lets the copies drain without stalling the PE.
                xT_sb = xpool.tile([P, H, NQC], BF, tag="x")
                xTs[kc] = xT_sb
                # exp split by q-half: the hf0-half exps fire mid-chunk
                # (right after their adds), so only two small hf1-half exps
                # remain on the chunk-boundary chain.
                for hf in range(2):
                    qs = slice(hf * 64, (hf + 1) * 64)
                    for hg in range(2):      # head groups of 4
                        hh = slice(hg * 4, (hg + 1) * 4)
                        nc.scalar.activation(xT_sb[:, hh, qs],
                                             e_sb[:, hh, qs],
                                             mybir.ActivationFunctionType.Exp)
                if kc + 1 < kc_n:
                    v_build(kc + 1)
                if kc + 2 < kc_n:
                    st_build(kc + 2)
            pv_emit(kc_n - 1)

            # ---- epilogue: normalize, transpose, output projection ----
            recip_sb = const.tile([P, H], FP)
            nc.vector.reciprocal(
                recip_sb, o_ps.rearrange("p (h e) -> p h e", h=H)[:, :, D])
            o_sb = const.tile([P, 2, P], FP)     # [q, half, (h d)%128]
            ov = o_ps.rearrange("p (h e) -> p h e", h=H)
            rv = bass.AP(tensor=recip_sb.tensor, offset=recip_sb.offset,
                         ap=[list(recip_sb.ap[0])]
                         + [[recip_sb.ap[1][0], 4], [0, D]])
            for half in range(2):
                nc.vector.tensor_tensor(
                    o_sb[:, half, :].rearrange("p (h d) -> p h d", h=4),
                    ov[:, half * 4:(half + 1) * 4, 0:D],
                    bass.AP(tensor=rv.tensor,
                            offset=rv.offset + half * 4 * recip_sb.ap[1][0],
                            ap=rv.ap), mult)
            oT_sb = const.tile([P, 2, P], FP)
            for m in range(2):
                t_full = proj_ps.tile([P, 512], FP, tag="proj", name="t_full")
                t_ps = t_full[:, :P]
                nc.tensor.transpose(t_ps, o_sb[:, m, :], ident)
                nc.vector.tensor_copy(oT_sb[:, m, :], t_ps)
            ps = proj_ps.tile([P, 512], FP, tag="proj")
            for m in range(2):
                nc.tensor.matmul(ps[:, :CQ], lhsT=oT_sb[:, m, :],
                                 rhs=wp_sb[:, m, :], start=(m == 0), stop=False)
            nc.tensor.matmul(ps[:, :CQ], lhsT=ones_row, rhs=bp_sb,
                             start=False, stop=True)
            y_sb = const.tile([P, CQ], FP)
            nc.vector.tensor_copy(y_sb, ps[:, :CQ])
            nc.sync.dma_start(y[:], y_sb)

    nc.compile()
    return nc


def prep_inputs(x_q, x_kv, z, Wq, bq, Wkv, bkv, Wb, bb, Wp, bp, nk=1024):
    """Host-side shard prep.  Returns in_maps for the 8 cores."""
    kc_n = nk // P
    gpc = NQC // TQ
    xkvT = np.ascontiguousarray(x_kv[0].T).astype(NP_BF)     # [CKV, nk]
    # bkv_V folds into bp exactly: softmax rows sum to 1, so the +bkvV on V
    # adds bkvV @ Wp to every output row.  bb / bkv_K cancel in softmax.
    bp_eff = (np.asarray(bp, dtype=np.float64)
              + np.asarray(bkv[H * D:], dtype=np.float64)
              @ np.asarray(Wp, dtype=np.float64)).astype(np.float32)
    shared = dict(xkvT=xkvT,
                  Wq=np.ascontiguousarray(Wq).astype(NP_BF),
                  bq=np.ascontiguousarray(bq, dtype=np.float32),
                  Wkv=np.ascontiguousarray(Wkv).astype(NP_BF),
                  Wb=np.ascontiguousarray(Wb).astype(NP_BF),
                  Wp=np.ascontiguousarray(Wp, dtype=np.float32),
                  bp=bp_eff)
    in_maps = []
    for i in range(NCORES):
        qs = i * NQC
        zi = z[0, qs:qs + NQC]                           # [q, k, c]
        # -> [g=(kc,qb), c, tq, k] with q = qb*TQ + tq, k = kc*128 + k
        zi = (zi.reshape(gpc, TQ, kc_n, P, BD)
              .transpose(2, 0, 4, 1, 3)                  # [kc, qb, c, tq, k]
              .reshape(kc_n * gpc, BD, TQ, P))
        in_maps.append(dict(
            zT=np.ascontiguousarray(zi).astype(NP_F8),
            xqT=np.ascontiguousarray(x_q[0, qs:qs + NQC].T).astype(NP_BF),
            **shared,
        ))
    return in_maps


_NC_CACHE = {}


def kernel(x_q, x_kv, z, Wq, bq, Wkv, bkv, Wb, bb, Wp, bp):
    key = "full"
    if key not in _NC_CACHE:
        _NC_CACHE[key] = build_program()
    nc = _NC_CACHE[key]
    in_maps = prep_inputs(x_q, x_kv, z, Wq, bq, Wkv, bkv, Wb, bb, Wp, bp)
    res = run_bass_kernel_spmd(nc, in_maps, list(range(NCORES)))
    out = np.empty((1, NQ, CQ), dtype=np.float32)
    for i in range(NCORES):
        out[0, i * NQC:(i + 1) * NQC, :] = res.results[i]["y"]
    return out


# revision 49
# speedup vs baseline: 1.0252x; 1.0252x over previous
"""BiasAttention TRN2 kernel — q-sharded across 8 NeuronCores.

Each core owns a block of 128 queries and computes the full attention for
them (all 8 heads, all 1024 keys), including the z-bias projection, with no
collectives.  The whole attention is computed TRANSPOSED (S^T[k,q],
bias^T[k,q]) so the exp'd scores feed the PV matmul directly with no PE
transposes.  Host-side prep lays z out per core as [g, c, tq, k] (contract
dim c on partitions, one query's [c,k] tile per stationary) and casts z to
fp8e4m3 — halving the dominant HBM stream; Wb stays bf16 so the
quantization error is z-only.  Exact-math folds: bb and the K-side bkv
bias are constant along the softmax axis (cancel), and the V-side bkv bias
folds into bp on host (softmax rows sum to 1).
"""

import sys

if "/opt/trn_rl_repo" not in sys.path:
    sys.path.insert(0, "/opt/trn_rl_repo")

import ml_dtypes
import numpy as np

import concourse.bass as bass
import concourse.mybir as mybir
from concourse import bacc
from concourse.bass_utils import run_bass_kernel_spmd
from concourse.masks import make_identity
from concourse.tile import TileContext

P = 128          # partitions
H = 8            # heads
D = 32           # head dim
CQ = 256         # q channels
CKV = 256        # kv channels
BD = 128         # bias (z) channels
NQ = 1024        # total queries
NCORES = 8
NQC = NQ // NCORES   # queries per core = 128
SCALE = D ** (-0.5)

TQ = 32          # queries per z DMA group (tile = [c, TQ, 128k], 4KB/part)
FP = mybir.dt.float32
BF = mybir.dt.bfloat16
F8 = mybir.dt.float8e4
NP_BF = ml_dtypes.bfloat16
NP_F8 = ml_dtypes.float8_e4m3


def build_program(nk=1024):
    kc_n = nk // P            # k-chunks of 128
    gpc = NQC // TQ           # z groups per k-chunk (4)
    ng = kc_n * gpc           # z DMA groups (32)
    add = mybir.AluOpType.add
    mult = mybir.AluOpType.mult

    nc = bacc.Bacc("TRN2", target_bir_lowering=False, debug=False,
                   num_devices=NCORES)

    # ---- I/O ----
    zT = nc.dram_tensor("zT", [ng, BD, TQ, P], F8, kind="ExternalInput")
    xqT = nc.dram_tensor("xqT", [CQ, NQC], BF, kind="ExternalInput")
    xkvT = nc.dram_tensor("xkvT", [CKV, nk], BF, kind="ExternalInput")
    Wq = nc.dram_tensor("Wq", [CQ, H * D], BF, kind="ExternalInput")
    bq = nc.dram_tensor("bq", [H * D], FP, kind="ExternalInput")
    Wkv = nc.dram_tensor("Wkv", [CKV, 2 * H * D], BF, kind="ExternalInput")
    Wb = nc.dram_tensor("Wb", [BD, H], BF, kind="ExternalInput")
    Wp = nc.dram_tensor("Wp", [H * D, CQ], FP, kind="ExternalInput")
    bp = nc.dram_tensor("bp", [CQ], FP, kind="ExternalInput")
    y = nc.dram_tensor("y", [NQC, CQ], FP, kind="ExternalOutput")

    with TileContext(nc) as tc:
        with (
            tc.tile_pool(name="const", bufs=1) as const,
            tc.tile_pool(name="zpool", bufs=24) as zpool,
            tc.tile_pool(name="epool", bufs=6) as epool,
            tc.tile_pool(name="xpool", bufs=6) as xpool,
            tc.tile_pool(name="proj_ps", bufs=2, space="PSUM") as proj_ps,
            tc.tile_pool(name="stv_ps", bufs=2, space="PSUM") as stv_ps,
            tc.tile_pool(name="b_ps", bufs=3, space="PSUM") as b_psp,
            tc.tile_pool(name="o_ps", bufs=1, space="PSUM") as o_psp,
        ):
            # ---- z stream leads the sync queue; a stalled z trigger only
            # head-of-line blocks other z triggers (and the final y store).
            wb_sb = const.tile([P, H], BF)
            nc.sync.dma_start(wb_sb, Wb[:])
            zs = {}
            for g in range(6):
                z_sb = zpool.tile([P, TQ, P], F8, tag="z", name=f"z{g}")
                nc.sync.dma_start(z_sb, zT[g])
                zs[g] = z_sb

            def z_fetch(g):
                if g not in zs:
                    z_sb = zpool.tile([P, TQ, P], F8, tag="z")
                    nc.sync.dma_start(z_sb, zT[g])
                    zs[g] = z_sb
                return zs[g]

            # ---- consts: big ones on the scalar HWDGE queue, tiny on
            # gpsimd software DGE.
            wq_sb = const.tile([P, 2, H * D], BF)
            nc.scalar.dma_start(wq_sb, Wq.rearrange("(o p) m -> p o m", p=P))
            xqT_sb = const.tile([P, 2, NQC], BF)
            nc.scalar.dma_start(xqT_sb, xqT.rearrange("(o p) q -> p o q", p=P))
            wkv_sb = const.tile([P, 2, 2 * H * D], BF)
            nc.scalar.dma_start(wkv_sb, Wkv.rearrange("(o p) m -> p o m", p=P))
            xkvT_sb = const.tile([P, 2, nk], BF)
            nc.scalar.dma_start(xkvT_sb, xkvT.rearrange("(o p) k -> p o k", p=P))
            wp_sb = const.tile([P, 2, CQ], FP)
            nc.scalar.dma_start(wp_sb, Wp.rearrange("(o p) m -> p o m", p=P))
            bq_sb = const.tile([P, 2], FP)
            nc.gpsimd.dma_start(bq_sb, bq.rearrange("(o p) -> p o", p=P))
            bp_sb = const.tile([1, CQ], FP)
            nc.gpsimd.dma_start(bp_sb, bp[None, :])
            ident = const.tile([P, P], FP)
            make_identity(nc, ident)
            ones_row = const.tile([1, P], FP)
            nc.vector.memset(ones_row, 1.0)

            # V augmented with a ones column per head: [k, kc, h, D+1]
            vaug_sb = const.tile([P, kc_n, H, D + 1], BF)
            nc.vector.memset(vaug_sb, 1.0)

            # ---- projections (bf16 in, fp32 psum accumulate) ----
            # Q^T [(h d), q] with (x + bq) * SCALE folded in, stored bf16
            qT_sb = const.tile([P, 2, NQC], BF)
            for m in range(2):
                ps = proj_ps.tile([P, 512], FP, tag="proj")
                for c in range(2):
                    nc.tensor.matmul(ps[:, :NQC],
                                     lhsT=wq_sb[:, c, m * P:(m + 1) * P],
                                     rhs=xqT_sb[:, c, :],
                                     start=(c == 0), stop=(c == 1))
                nc.vector.tensor_scalar(qT_sb[:, m, :], ps[:, :NQC],
                                        bq_sb[:, m:m + 1], SCALE, add, mult)

            # K^T [(h d), k] — bkv_K is softmax-invariant, dropped
            kT_sb = const.tile([P, 2, nk], BF)
            for m in range(2):
                for nh in range((nk + 511) // 512):
                    nn_ = min(512, nk - nh * 512)
                    ps = proj_ps.tile([P, 512], FP, tag="proj")
                    for c in range(2):
                        nc.tensor.matmul(ps[:, :nn_],
                                         lhsT=wkv_sb[:, c, m * P:(m + 1) * P],
                                         rhs=xkvT_sb[:, c, nh * 512:nh * 512 + nn_],
                                         start=(c == 0), stop=(c == 1))
                    if (m * 2 + nh) % 2 == 0:
                        nc.scalar.activation(
                            kT_sb[:, m, nh * 512:nh * 512 + nn_], ps[:, :nn_],
                            mybir.ActivationFunctionType.Copy)
                    else:
                        nc.vector.tensor_copy(
                            kT_sb[:, m, nh * 512:nh * 512 + nn_], ps[:, :nn_])

            # qT/kT reshuffled so every head's d-dim sits on partitions
            # 0-31: the S^T matmuls then run at default tile position and
            # can share psum banks at different column offsets (the same
            # pattern the z matmuls use) — tile-positioned matmuls sharing
            # a bank wedge the device.  One SBUF->SBUF DMA per head.
            qT32 = const.tile([32, H, NQC], BF)
            kT32 = const.tile([32, H, nk], BF)
            for hq in range(4):
                nc.gpsimd.dma_start(qT32[:, hq::4, :],
                                    qT_sb[hq * 32:(hq + 1) * 32, :, :])
            for m in range(2):
                for hq in range(4):
                    nc.scalar.dma_start(kT32[:, m * 4 + hq, :],
                                        kT_sb[hq * 32:(hq + 1) * 32, m, :])

            # S^T[k, kc, h, q]: per (kc, head) a [k=128, q=128] matmul;
            # 4 heads batch into one psum tile with a single copy out.
            sT_sb = const.tile([P, kc_n, H, NQC], FP)

            def st_build(kc):
                for hg in range(2):
                    ps = stv_ps.tile([P, 512], FP, tag="stv", name="st_ps")
                    for hl in range(4):
                        h = hg * 4 + hl
                        nc.tensor.matmul(ps[:, hl * P:(hl + 1) * P],
                                         lhsT=kT32[:, h, kc * P:(kc + 1) * P],
                                         rhs=qT32[:, h, :],
                                         start=True, stop=True)
                    nc.vector.tensor_copy(
                        sT_sb[:, kc, hg * 4:(hg + 1) * 4, :],
                        ps.rearrange("p (hl q) -> p hl q", hl=4))

            # V [k, (h d)] (bias folded into bp on host) into vaug
            def v_build(kc):
                ps = stv_ps.tile([P, 512], FP, tag="stv", name="v_ps")
                for c in range(2):
                    nc.tensor.matmul(ps[:, :H * D],
                                     lhsT=xkvT_sb[:, c, kc * P:(kc + 1) * P],
                                     rhs=wkv_sb[:, c, H * D:2 * H * D],
                                     start=(c == 0), stop=(c == 1))
                nc.vector.tensor_copy(
                    vaug_sb[:, kc, :, 0:D],
                    ps[:, :H * D].rearrange("p (h d) -> p h d", h=H))

            st_build(0)
            st_build(1)
            v_build(0)

            # ---- main loop over k-chunks, software-pipelined: PV for chunk
            # kc is emitted after the z matmuls of chunk kc+1, so by the
            # time the PE reaches it the exp has long finished (no PE stall
            # on the cross-engine add->exp chain).
            o_ps = o_psp.tile([P, H * (D + 1)], FP)   # [q, h*(D+1)]
            xTs = {}

            def pv_emit(kc):
                xT_sb = xTs.pop(kc)
                for h in range(H):
                    # o_ps lives in one bank: open the accumulation group
                    # on the first matmul only, close on the last.
                    nc.tensor.matmul(
                        o_ps[:, h * (D + 1):(h + 1) * (D + 1)],
                        lhsT=xT_sb[:, h, :], rhs=vaug_sb[:, kc, h, :],
                        start=(kc == 0 and h == 0),
                        stop=(kc == kc_n - 1 and h == H - 1))

            for kc in range(kc_n):
                # e^T holds the chunk's logits [k, h, q] (bf16)
                e_sb = epool.tile([P, H, NQC], BF, tag="e")
                bps = []
                for hf in range(2):
                    # bias^T: one [k, 8] matmul per query into b_ps
                    # [k, (q64 h8)]; 64 queries per psum bank.
                    b_ps = b_psp.tile([P, 64 * H], FP, tag="b")
                    bps.append(b_ps)
                    for qb in (2 * hf, 2 * hf + 1):
                        z_sb = z_fetch(kc * gpc + qb)
                        for t in range(TQ):
                            ql = (qb % 2) * TQ + t
                            nc.tensor.matmul(b_ps[:, ql * H:(ql + 1) * H],
                                             lhsT=z_sb[:, t, :], rhs=wb_sb,
                                             start=(ql == 0),
                                             stop=(ql == 63))
                # adds first (critical chain), then PV two chunks back (its
                # exp is long done), then the V / S^T prefetch builds.
                for hf in range(2):
                    bv = bps[hf].rearrange("p (q h) -> p h q", h=H)
                    qs = slice(hf * 64, (hf + 1) * 64)
                    for hg in range(2):
                        hh = slice(hg * 4, (hg + 1) * 4)
                        nc.vector.tensor_tensor(
                            e_sb[:, hh, qs], sT_sb[:, kc, hh, qs],
                            bv[:, hh, :], add)
                if kc - 2 in xTs:
                    pv_emit(kc - 2)
                if kc == kc_n - 1:
                    # drain the PV backlog: PV(kc-1)'s exp completed during
                    # this chunk's z matmuls, so it is stall-free here
                    pv_emit(kc - 1)
                # exps BEFORE the v/st builds: their psum copies queue
                # behind the exps on ACT, and the next chunk's v/st matmuls
                # wait on those copies through the stv ring — emitting exps
                # first lets the copies drain without stalling the PE.
                xT_sb = xpool.tile([P, H, NQC], BF, tag="x")
                xTs[kc] = xT_sb
                for hg in range(2):          # head groups of 4
                    hh = slice(hg * 4, (hg + 1) * 4)
                    nc.scalar.activation(xT_sb[:, hh, :], e_sb[:, hh, :],
                                         mybir.ActivationFunctionType.Exp)
                if kc + 1 < kc_n:
                    v_build(kc + 1)
                if kc + 2 < kc_n:
                    st_build(kc + 2)
            pv_emit(kc_n - 1)

            # ---- epilogue: normalize, transpose, output projection ----
            recip_sb = const.tile([P, H], FP)
            nc.vector.reciprocal(
                recip_sb, o_ps.rearrange("p (h e) -> p h e", h=H)[:, :, D])
            o_sb = const.tile([P, 2, P], FP)     # [q, half, (h d)%128]
            ov = o_ps.rearrange("p (h e) -> p h e", h=H)
            rv = bass.AP(tensor=recip_sb.tensor, offset=recip_sb.offset,
                         ap=[list(recip_sb.ap[0])]
                         + [[recip_sb.ap[1][0], 4], [0, D]])
            for half in range(2):
                nc.vector.tensor_tensor(
                    o_sb[:, half, :].rearrange("p (h d) -> p h d", h=4),
                    ov[:, half * 4:(half + 1) * 4, 0:D],
                    bass.AP(tensor=rv.tensor,
                            offset=rv.offset + half * 4 * recip_sb.ap[1][0],
                            ap=rv.ap), mult)
            oT_sb = const.tile([P, 2, P], FP)
            for m in range(2):
                t_full = proj_ps.tile([P, 512], FP, tag="proj", name="t_full")
                t_ps = t_full[:, :P]
                nc.tensor.transpose(t_ps, o_sb[:, m, :], ident)
                nc.vector.tensor_copy(oT_sb[:, m, :], t_ps)
            ps = proj_ps.tile([P, 512], FP, tag="proj")
            for m in range(2):
                nc.tensor.matmul(ps[:, :CQ], lhsT=oT_sb[:, m, :],
                                 rhs=wp_sb[:, m, :], start=(m == 0), stop=False)
            nc.tensor.matmul(ps[:, :CQ], lhsT=ones_row, rhs=bp_sb,
                             start=False, stop=True)
            y_sb = const.tile([P, CQ], FP)
            nc.vector.tensor_copy(y_sb, ps[:, :CQ])
            nc.sync.dma_start(y[:], y_sb)

    nc.compile()
    return nc


def prep_inputs(x_q, x_kv, z, Wq, bq, Wkv, bkv, Wb, bb, Wp, bp, nk=1024):
    """Host-side shard prep.  Returns in_maps for the 8 cores."""
    kc_n = nk // P
    gpc = NQC // TQ
    xkvT = np.ascontiguousarray(x_kv[0].T).astype(NP_BF)     # [CKV, nk]
    # bkv_V folds into bp exactly: softmax rows sum to 1, so the +bkvV on V
    # adds bkvV @ Wp to every output row.  bb / bkv_K cancel in softmax.
    bp_eff = (np.asarray(bp, dtype=np.float64)
              + np.asarray(bkv[H * D:], dtype=np.float64)
              @ np.asarray(Wp, dtype=np.float64)).astype(np.float32)
    shared = dict(xkvT=xkvT,
                  Wq=np.ascontiguousarray(Wq).astype(NP_BF),
                  bq=np.ascontiguousarray(bq, dtype=np.float32),
                  Wkv=np.ascontiguousarray(Wkv).astype(NP_BF),
                  Wb=np.ascontiguousarray(Wb).astype(NP_BF),
                  Wp=np.ascontiguousarray(Wp, dtype=np.float32),
                  bp=bp_eff)
    in_maps = []
    for i in range(NCORES):
        qs = i * NQC
        zi = z[0, qs:qs + NQC]                           # [q, k, c]
        # -> [g=(kc,qb), c, tq, k] with q = qb*TQ + tq, k = kc*128 + k
        zi = (zi.reshape(gpc, TQ, kc_n, P, BD)
              .transpose(2, 0, 4, 1, 3)                  # [kc, qb, c, tq, k]
              .reshape(kc_n * gpc, BD, TQ, P))
        in_maps.append(dict(
            zT=np.ascontiguousarray(zi).astype(NP_F8),
            xqT=np.ascontiguousarray(x_q[0, qs:qs + NQC].T).astype(NP_BF),
            **shared,
        ))
    return in_maps


_NC_CACHE = {}


def kernel(x_q, x_kv, z, Wq, bq, Wkv, bkv, Wb, bb, Wp, bp):
    key = "full"
    if key not in _NC_CACHE:
        _NC_CACHE[key] = build_program()
    nc = _NC_CACHE[key]
    in_maps = prep_inputs(x_q, x_kv, z, Wq, bq, Wkv, bkv, Wb, bb, Wp, bp)
    res = run_bass_kernel_spmd(nc, in_maps, list(range(NCORES)))
    out = np.empty((1, NQ, CQ), dtype=np.float32)
    for i in range(NCORES):
        out[0, i * NQC:(i + 1) * NQC, :] = res.results[i]["y"]
    return out


# revision 53
# speedup vs baseline: 1.1134x; 1.0860x over previous
"""BiasAttention TRN2 kernel — q-sharded across 8 NeuronCores.

Each core owns a block of 128 queries and computes the full attention for
them (all 8 heads, all 1024 keys), including the z-bias projection, with no
collectives.  The whole attention is computed TRANSPOSED (S^T[k,q],
bias^T[k,q]) so the exp'd scores feed the PV matmul directly with no PE
transposes.  Host-side prep lays z out per core as [g, c, tq, k] (contract
dim c on partitions, one query's [c,k] tile per stationary) and casts z to
fp8e4m3 — halving the dominant HBM stream; Wb stays bf16 so the
quantization error is z-only.  Exact-math folds: bb and the K-side bkv
bias are constant along the softmax axis (cancel), and the V-side bkv bias
folds into bp on host (softmax rows sum to 1).
"""

import sys

if "/opt/trn_rl_repo" not in sys.path:
    sys.path.insert(0, "/opt/trn_rl_repo")

import ml_dtypes
import numpy as np

import concourse.bass as bass
import concourse.mybir as mybir
from concourse import bacc
from concourse.bass_utils import run_bass_kernel_spmd
from concourse.masks import make_identity
from concourse.tile import TileContext

P = 128          # partitions
H = 8            # heads
D = 32           # head dim
CQ = 256         # q channels
CKV = 256        # kv channels
BD = 128         # bias (z) channels
NQ = 1024        # total queries
NCORES = 8
NQC = NQ // NCORES   # queries per core = 128
SCALE = D ** (-0.5)

TQ = 32          # queries per z DMA group (tile = [c, TQ, 128k], 4KB/part)
FP = mybir.dt.float32
BF = mybir.dt.bfloat16
F8 = mybir.dt.float8e4
NP_BF = ml_dtypes.bfloat16
NP_F8 = ml_dtypes.float8_e4m3


def build_program(nk=1024):
    kc_n = nk // P            # k-chunks of 128
    gpc = NQC // TQ           # z groups per k-chunk (4)
    ng = kc_n * gpc           # z DMA groups (32)
    add = mybir.AluOpType.add
    mult = mybir.AluOpType.mult

    nc = bacc.Bacc("TRN2", target_bir_lowering=False, debug=False,
                   num_devices=NCORES)

    # ---- I/O ----
    zT = nc.dram_tensor("zT", [ng, BD, TQ, P], F8, kind="ExternalInput")
    xqT = nc.dram_tensor("xqT", [CQ, NQC], BF, kind="ExternalInput")
    xkvT = nc.dram_tensor("xkvT", [CKV, nk], BF, kind="ExternalInput")
    Wq = nc.dram_tensor("Wq", [CQ, H * D], BF, kind="ExternalInput")
    bq = nc.dram_tensor("bq", [H * D], FP, kind="ExternalInput")
    Wkv = nc.dram_tensor("Wkv", [CKV, 2 * H * D], BF, kind="ExternalInput")
    Wb = nc.dram_tensor("Wb", [BD, H], BF, kind="ExternalInput")
    Wp = nc.dram_tensor("Wp", [H * D, CQ], FP, kind="ExternalInput")
    bp = nc.dram_tensor("bp", [CQ], FP, kind="ExternalInput")
    y = nc.dram_tensor("y", [NQC, CQ], FP, kind="ExternalOutput")

    with TileContext(nc) as tc:
        with (
            tc.tile_pool(name="const", bufs=1) as const,
            tc.tile_pool(name="zpool", bufs=28) as zpool,
            tc.tile_pool(name="epool", bufs=6) as epool,
            tc.tile_pool(name="xpool", bufs=6) as xpool,
            tc.tile_pool(name="proj_ps", bufs=2, space="PSUM") as proj_ps,
            tc.tile_pool(name="stv_ps", bufs=2, space="PSUM") as stv_ps,
            tc.tile_pool(name="b_ps", bufs=3, space="PSUM") as b_psp,
            tc.tile_pool(name="o_ps", bufs=1, space="PSUM") as o_psp,
        ):
            # ---- z stream leads the sync queue; a stalled z trigger only
            # head-of-line blocks other z triggers (and the final y store).
            wb_sb = const.tile([P, H], BF)
            nc.sync.dma_start(wb_sb, Wb[:])
            zs = {}
            for g in range(6):
                z_sb = zpool.tile([P, TQ, P], F8, tag="z", name=f"z{g}")
                nc.sync.dma_start(z_sb, zT[g])
                zs[g] = z_sb

            def z_fetch(g):
                if g not in zs:
                    z_sb = zpool.tile([P, TQ, P], F8, tag="z")
                    nc.sync.dma_start(z_sb, zT[g])
                    zs[g] = z_sb
                return zs[g]

            # ---- consts: big ones on the scalar HWDGE queue, tiny on
            # gpsimd software DGE.
            # K-side consts first: they gate the longest prologue chain
            # (K-proj -> kT32 shuffle -> S^T -> first adds).
            wkv_sb = const.tile([P, 2, 2 * H * D], BF)
            nc.scalar.dma_start(wkv_sb, Wkv.rearrange("(o p) m -> p o m", p=P))
            xkvT_sb = const.tile([P, 2, nk], BF)
            nc.scalar.dma_start(xkvT_sb, xkvT.rearrange("(o p) k -> p o k", p=P))
            wq_sb = const.tile([P, 2, H * D], BF)
            nc.scalar.dma_start(wq_sb, Wq.rearrange("(o p) m -> p o m", p=P))
            xqT_sb = const.tile([P, 2, NQC], BF)
            nc.scalar.dma_start(xqT_sb, xqT.rearrange("(o p) q -> p o q", p=P))
            wp_sb = const.tile([P, 2, CQ], FP)
            nc.scalar.dma_start(wp_sb, Wp.rearrange("(o p) m -> p o m", p=P))
            bq_sb = const.tile([P, 2], FP)
            nc.gpsimd.dma_start(bq_sb, bq.rearrange("(o p) -> p o", p=P))
            bp_sb = const.tile([1, CQ], FP)
            nc.gpsimd.dma_start(bp_sb, bp[None, :])
            ident = const.tile([P, P], FP)
            make_identity(nc, ident)
            ones_row = const.tile([1, P], FP)
            nc.vector.memset(ones_row, 1.0)

            # V augmented with a ones column per head: [k, kc, h, D+1]
            vaug_sb = const.tile([P, kc_n, H, D + 1], BF)
            nc.vector.memset(vaug_sb, 1.0)

            # ---- projections (bf16 in, fp32 psum accumulate) ----
            # Q^T [(h d), q] with (x + bq) * SCALE folded in, stored bf16
            qT_sb = const.tile([P, 2, NQC], BF)
            for m in range(2):
                ps = proj_ps.tile([P, 512], FP, tag="proj")
                for c in range(2):
                    nc.tensor.matmul(ps[:, :NQC],
                                     lhsT=wq_sb[:, c, m * P:(m + 1) * P],
                                     rhs=xqT_sb[:, c, :],
                                     start=(c == 0), stop=(c == 1))
                nc.vector.tensor_scalar(qT_sb[:, m, :], ps[:, :NQC],
                                        bq_sb[:, m:m + 1], SCALE, add, mult)

            # K^T [(h d), k] — bkv_K is softmax-invariant, dropped
            kT_sb = const.tile([P, 2, nk], BF)
            for m in range(2):
                for nh in range((nk + 511) // 512):
                    nn_ = min(512, nk - nh * 512)
                    ps = proj_ps.tile([P, 512], FP, tag="proj")
                    for c in range(2):
                        nc.tensor.matmul(ps[:, :nn_],
                                         lhsT=wkv_sb[:, c, m * P:(m + 1) * P],
                                         rhs=xkvT_sb[:, c, nh * 512:nh * 512 + nn_],
                                         start=(c == 0), stop=(c == 1))
                    if (m * 2 + nh) % 2 == 0:
                        nc.scalar.activation(
                            kT_sb[:, m, nh * 512:nh * 512 + nn_], ps[:, :nn_],
                            mybir.ActivationFunctionType.Copy)
                    else:
                        nc.vector.tensor_copy(
                            kT_sb[:, m, nh * 512:nh * 512 + nn_], ps[:, :nn_])

            # qT/kT reshuffled so every head's d-dim sits on partitions
            # 0-31: the S^T matmuls then run at default tile position and
            # can share psum banks at different column offsets (the same
            # pattern the z matmuls use) — tile-positioned matmuls sharing
            # a bank wedge the device.  One SBUF->SBUF DMA per head.
            qT32 = const.tile([32, H, NQC], BF)
            kT32 = const.tile([32, H, nk], BF)
            for hq in range(4):
                nc.gpsimd.dma_start(qT32[:, hq::4, :],
                                    qT_sb[hq * 32:(hq + 1) * 32, :, :])
            for m in range(2):
                for hq in range(4):
                    nc.scalar.dma_start(kT32[:, m * 4 + hq, :],
                                        kT_sb[hq * 32:(hq + 1) * 32, m, :])

            # S^T[k, kc, h, q]: per (kc, head) a [k=128, q=128] matmul;
            # 4 heads batch into one psum tile with a single copy out.
            sT_sb = const.tile([P, kc_n, H, NQC], BF)

            def st_build(kc):
                for hg in range(2):
                    ps = stv_ps.tile([P, 512], FP, tag="stv", name="st_ps")
                    for hl in range(4):
                        h = hg * 4 + hl
                        nc.tensor.matmul(ps[:, hl * P:(hl + 1) * P],
                                         lhsT=kT32[:, h, kc * P:(kc + 1) * P],
                                         rhs=qT32[:, h, :],
                                         start=True, stop=True)
                    nc.vector.tensor_copy(
                        sT_sb[:, kc, hg * 4:(hg + 1) * 4, :],
                        ps.rearrange("p (hl q) -> p hl q", hl=4))

            # V [k, (h d)] (bias folded into bp on host) into vaug
            def v_build(kc):
                ps = stv_ps.tile([P, 512], FP, tag="stv", name="v_ps")
                for c in range(2):
                    nc.tensor.matmul(ps[:, :H * D],
                                     lhsT=xkvT_sb[:, c, kc * P:(kc + 1) * P],
                                     rhs=wkv_sb[:, c, H * D:2 * H * D],
                                     start=(c == 0), stop=(c == 1))
                nc.vector.tensor_copy(
                    vaug_sb[:, kc, :, 0:D],
                    ps[:, :H * D].rearrange("p (h d) -> p h d", h=H))

            st_build(0)
            st_build(1)
            v_build(0)

            # ---- main loop over k-chunks, software-pipelined: PV for chunk
            # kc is emitted after the z matmuls of chunk kc+1, so by the
            # time the PE reaches it the exp has long finished (no PE stall
            # on the cross-engine add->exp chain).
            o_ps = o_psp.tile([P, H * (D + 1)], FP)   # [q, h*(D+1)]
            xTs = {}

            def pv_emit(kc):
                xT_sb = xTs.pop(kc)
                for h in range(H):
                    # o_ps lives in one bank: open the accumulation group
                    # on the first matmul only, close on the last.
                    nc.tensor.matmul(
                        o_ps[:, h * (D + 1):(h + 1) * (D + 1)],
                        lhsT=xT_sb[:, h, :], rhs=vaug_sb[:, kc, h, :],
                        start=(kc == 0 and h == 0),
                        stop=(kc == kc_n - 1 and h == H - 1))

            for kc in range(kc_n):
                # e^T holds the chunk's logits [k, h, q] (bf16)
                e_sb = epool.tile([P, H, NQC], BF, tag="e")
                bps = []
                for hf in range(2):
                    # bias^T: one [k, 8] matmul per query into b_ps
                    # [k, (q64 h8)]; 64 queries per psum bank.
                    b_ps = b_psp.tile([P, 64 * H], FP, tag="b")
                    bps.append(b_ps)
                    for qb in (2 * hf, 2 * hf + 1):
                        z_sb = z_fetch(kc * gpc + qb)
                        for t in range(TQ):
                            ql = (qb % 2) * TQ + t
                            # per-32-query accumulation groups: the first
                            # half's adds fire a full z-group earlier, and
                            # the boundary add waits a quarter-group drain
                            nc.tensor.matmul(b_ps[:, ql * H:(ql + 1) * H],
                                             lhsT=z_sb[:, t, :], rhs=wb_sb,
                                             start=(t == 0),
                                             stop=(t == TQ - 1))
                # adds first (critical chain), then PV two chunks back (its
                # exp is long done), then the V / S^T prefetch builds.
                for hf in range(2):
                    bv = bps[hf].rearrange("p (q h) -> p h q", h=H)
                    for qh in range(2):
                        qs = slice(hf * 64 + qh * 32, hf * 64 + (qh + 1) * 32)
                        for hg in range(2):
                            hh = slice(hg * 4, (hg + 1) * 4)
                            nc.vector.tensor_tensor(
                                e_sb[:, hh, qs], sT_sb[:, kc, hh, qs],
                                bv[:, hh, qh * 32:(qh + 1) * 32], add)
                if kc - 2 in xTs:
                    pv_emit(kc - 2)
                if kc == kc_n - 1:
                    # drain the PV backlog: PV(kc-1)'s exp completed during
                    # this chunk's z matmuls, so it is stall-free here
                    pv_emit(kc - 1)
                # exps BEFORE the v/st builds: their psum copies queue
                # behind the exps on ACT, and the next chunk's v/st matmuls
                # wait on those copies through the stv ring — emitting exps
                # first lets the copies drain without stalling the PE.
                xT_sb = xpool.tile([P, H, NQC], BF, tag="x")
                xTs[kc] = xT_sb
                for hg in range(2):          # head groups of 4
                    hh = slice(hg * 4, (hg + 1) * 4)
                    nc.scalar.activation(xT_sb[:, hh, :], e_sb[:, hh, :],
                                         mybir.ActivationFunctionType.Exp)
                if kc + 1 < kc_n:
                    v_build(kc + 1)
                if kc + 2 < kc_n:
                    st_build(kc + 2)
            pv_emit(kc_n - 1)

            # ---- epilogue: normalize, transpose, output projection ----
            recip_sb = const.tile([P, H], FP)
            nc.vector.reciprocal(
                recip_sb, o_ps.rearrange("p (h e) -> p h e", h=H)[:, :, D])
            o_sb = const.tile([P, 2, P], FP)     # [q, half, (h d)%128]
            ov = o_ps.rearrange("p (h e) -> p h e", h=H)
            rv = bass.AP(tensor=recip_sb.tensor, offset=recip_sb.offset,
                         ap=[list(recip_sb.ap[0])]
                         + [[recip_sb.ap[1][0], 4], [0, D]])
            for half in range(2):
                nc.vector.tensor_tensor(
                    o_sb[:, half, :].rearrange("p (h d) -> p h d", h=4),
                    ov[:, half * 4:(half + 1) * 4, 0:D],
                    bass.AP(tensor=rv.tensor,
                            offset=rv.offset + half * 4 * recip_sb.ap[1][0],
                            ap=rv.ap), mult)
            oT_sb = const.tile([P, 2, P], FP)
            for m in range(2):
                t_full = proj_ps.tile([P, 512], FP, tag="proj", name="t_full")
                t_ps = t_full[:, :P]
                nc.tensor.transpose(t_ps, o_sb[:, m, :], ident)
                nc.vector.tensor_copy(oT_sb[:, m, :], t_ps)
            ps = proj_ps.tile([P, 512], FP, tag="proj")
            for m in range(2):
                nc.tensor.matmul(ps[:, :CQ], lhsT=oT_sb[:, m, :],
                                 rhs=wp_sb[:, m, :], start=(m == 0), stop=False)
            nc.tensor.matmul(ps[:, :CQ], lhsT=ones_row, rhs=bp_sb,
                             start=False, stop=True)
            y_sb = const.tile([P, CQ], FP)
            nc.vector.tensor_copy(y_sb, ps[:, :CQ])
            nc.sync.dma_start(y[:], y_sb)

    nc.compile()
    return nc


def prep_inputs(x_q, x_kv, z, Wq, bq, Wkv, bkv, Wb, bb, Wp, bp, nk=1024):
    """Host-side shard prep.  Returns in_maps for the 8 cores."""
    kc_n = nk // P
    gpc = NQC // TQ
    xkvT = np.ascontiguousarray(x_kv[0].T).astype(NP_BF)     # [CKV, nk]
    # bkv_V folds into bp exactly: softmax rows sum to 1, so the +bkvV on V
    # adds bkvV @ Wp to every output row.  bb / bkv_K cancel in softmax.
    bp_eff = (np.asarray(bp, dtype=np.float64)
              + np.asarray(bkv[H * D:], dtype=np.float64)
              @ np.asarray(Wp, dtype=np.float64)).astype(np.float32)
    shared = dict(xkvT=xkvT,
                  Wq=np.ascontiguousarray(Wq).astype(NP_BF),
                  bq=np.ascontiguousarray(bq, dtype=np.float32),
                  Wkv=np.ascontiguousarray(Wkv).astype(NP_BF),
                  Wb=np.ascontiguousarray(Wb).astype(NP_BF),
                  Wp=np.ascontiguousarray(Wp, dtype=np.float32),
                  bp=bp_eff)
    in_maps = []
    for i in range(NCORES):
        qs = i * NQC
        zi = z[0, qs:qs + NQC]                           # [q, k, c]
        # -> [g=(kc,qb), c, tq, k] with q = qb*TQ + tq, k = kc*128 + k
        zi = (zi.reshape(gpc, TQ, kc_n, P, BD)
              .transpose(2, 0, 4, 1, 3)                  # [kc, qb, c, tq, k]
              .reshape(kc_n * gpc, BD, TQ, P))
        in_maps.append(dict(
            zT=np.ascontiguousarray(zi).astype(NP_F8),
            xqT=np.ascontiguousarray(x_q[0, qs:qs + NQC].T).astype(NP_BF),
            **shared,
        ))
    return in_maps


_NC_CACHE = {}


def kernel(x_q, x_kv, z, Wq, bq, Wkv, bkv, Wb, bb, Wp, bp):
    key = "full"
    if key not in _NC_CACHE:
        _NC_CACHE[key] = build_program()
    nc = _NC_CACHE[key]
    in_maps = prep_inputs(x_q, x_kv, z, Wq, bq, Wkv, bkv, Wb, bb, Wp, bp)
    res = run_bass_kernel_spmd(nc, in_maps, list(range(NCORES)))
    out = np.empty((1, NQ, CQ), dtype=np.float32)
    for i in range(NCORES):
        out[0, i * NQC:(i + 1) * NQC, :] = res.results[i]["y"]
    return out


# revision 54
# speedup vs baseline: 1.1823x; 1.0619x over previous
"""BiasAttention TRN2 kernel — q-sharded across 8 NeuronCores.

Each core owns a block of 128 queries and computes the full attention for
them (all 8 heads, all 1024 keys), including the z-bias projection, with no
collectives.  The whole attention is computed TRANSPOSED (S^T[k,q],
bias^T[k,q]) so the exp'd scores feed the PV matmul directly with no PE
transposes.  Host-side prep lays z out per core as [g, c, tq, k] (contract
dim c on partitions, one query's [c,k] tile per stationary) and casts z to
fp8e4m3 — halving the dominant HBM stream; Wb stays bf16 so the
quantization error is z-only.  Exact-math folds: bb and the K-side bkv
bias are constant along the softmax axis (cancel), and the V-side bkv bias
folds into bp on host (softmax rows sum to 1).
"""

import sys

if "/opt/trn_rl_repo" not in sys.path:
    sys.path.insert(0, "/opt/trn_rl_repo")

import ml_dtypes
import numpy as np

import concourse.bass as bass
import concourse.mybir as mybir
from concourse import bacc
from concourse.bass_utils import run_bass_kernel_spmd
from concourse.masks import make_identity
from concourse.tile import TileContext

P = 128          # partitions
H = 8            # heads
D = 32           # head dim
CQ = 256         # q channels
CKV = 256        # kv channels
BD = 128         # bias (z) channels
NQ = 1024        # total queries
NCORES = 8
NQC = NQ // NCORES   # queries per core = 128
SCALE = D ** (-0.5)

TQ = 32          # queries per z DMA group (tile = [c, TQ, 128k], 4KB/part)
FP = mybir.dt.float32
BF = mybir.dt.bfloat16
F8 = mybir.dt.float8e4
NP_BF = ml_dtypes.bfloat16
NP_F8 = ml_dtypes.float8_e4m3


def build_program(nk=1024):
    kc_n = nk // P            # k-chunks of 128
    gpc = NQC // TQ           # z groups per k-chunk (4)
    ng = kc_n * gpc           # z DMA groups (32)
    add = mybir.AluOpType.add
    mult = mybir.AluOpType.mult

    nc = bacc.Bacc("TRN2", target_bir_lowering=False, debug=False,
                   num_devices=NCORES)

    # ---- I/O ----
    zT = nc.dram_tensor("zT", [ng, BD, TQ, P], F8, kind="ExternalInput")
    xqT = nc.dram_tensor("xqT", [CQ, NQC], BF, kind="ExternalInput")
    xkvT = nc.dram_tensor("xkvT", [CKV, nk], BF, kind="ExternalInput")
    Wq = nc.dram_tensor("Wq", [CQ, H * D], BF, kind="ExternalInput")
    bq = nc.dram_tensor("bq", [H * D], FP, kind="ExternalInput")
    Wkv = nc.dram_tensor("Wkv", [CKV, 2 * H * D], BF, kind="ExternalInput")
    Wb = nc.dram_tensor("Wb", [BD, H], BF, kind="ExternalInput")
    Wp = nc.dram_tensor("Wp", [H * D, CQ], FP, kind="ExternalInput")
    bp = nc.dram_tensor("bp", [CQ], FP, kind="ExternalInput")
    y = nc.dram_tensor("y", [NQC, CQ], FP, kind="ExternalOutput")

    with TileContext(nc) as tc:
        with (
            tc.tile_pool(name="const", bufs=1) as const,
            tc.tile_pool(name="zpool", bufs=28) as zpool,
            tc.tile_pool(name="epool", bufs=6) as epool,
            tc.tile_pool(name="xpool", bufs=6) as xpool,
            tc.tile_pool(name="proj_ps", bufs=2, space="PSUM") as proj_ps,
            tc.tile_pool(name="stv_ps", bufs=2, space="PSUM") as stv_ps,
            tc.tile_pool(name="b_ps", bufs=3, space="PSUM") as b_psp,
            tc.tile_pool(name="o_ps", bufs=1, space="PSUM") as o_psp,
        ):
            # ---- z stream leads the sync queue; a stalled z trigger only
            # head-of-line blocks other z triggers (and the final y store).
            wb_sb = const.tile([P, H], BF)
            nc.sync.dma_start(wb_sb, Wb[:])
            zs = {}
            for g in range(6):
                z_sb = zpool.tile([P, TQ, P], F8, tag="z", name=f"z{g}")
                nc.sync.dma_start(z_sb, zT[g])
                zs[g] = z_sb

            def z_fetch(g):
                if g not in zs:
                    z_sb = zpool.tile([P, TQ, P], F8, tag="z")
                    nc.sync.dma_start(z_sb, zT[g])
                    zs[g] = z_sb
                return zs[g]

            # ---- consts: big ones on the scalar HWDGE queue, tiny on
            # gpsimd software DGE.
            wq_sb = const.tile([P, 2, H * D], BF)
            nc.scalar.dma_start(wq_sb, Wq.rearrange("(o p) m -> p o m", p=P))
            xqT_sb = const.tile([P, 2, NQC], BF)
            nc.scalar.dma_start(xqT_sb, xqT.rearrange("(o p) q -> p o q", p=P))
            wkv_sb = const.tile([P, 2, 2 * H * D], BF)
            nc.scalar.dma_start(wkv_sb, Wkv.rearrange("(o p) m -> p o m", p=P))
            xkvT_sb = const.tile([P, 2, nk], BF)
            nc.scalar.dma_start(xkvT_sb, xkvT.rearrange("(o p) k -> p o k", p=P))
            wp_sb = const.tile([P, 2, CQ], FP)
            nc.scalar.dma_start(wp_sb, Wp.rearrange("(o p) m -> p o m", p=P))
            bq_sb = const.tile([P, 2], FP)
            nc.gpsimd.dma_start(bq_sb, bq.rearrange("(o p) -> p o", p=P))
            bp_sb = const.tile([1, CQ], FP)
            nc.gpsimd.dma_start(bp_sb, bp[None, :])
            ident = const.tile([P, P], FP)
            make_identity(nc, ident)
            ones_row = const.tile([1, P], FP)
            nc.vector.memset(ones_row, 1.0)

            # V augmented with a ones column per head: [k, kc, h, D+1]
            vaug_sb = const.tile([P, kc_n, H, D + 1], BF)
            nc.vector.memset(vaug_sb, 1.0)

            # ---- projections (bf16 in, fp32 psum accumulate) ----
            # Q^T [(h d), q] with (x + bq) * SCALE folded in, stored bf16
            qT_sb = const.tile([P, 2, NQC], BF)
            for m in range(2):
                ps = proj_ps.tile([P, 512], FP, tag="proj")
                for c in range(2):
                    nc.tensor.matmul(ps[:, :NQC],
                                     lhsT=wq_sb[:, c, m * P:(m + 1) * P],
                                     rhs=xqT_sb[:, c, :],
                                     start=(c == 0), stop=(c == 1))
                nc.vector.tensor_scalar(qT_sb[:, m, :], ps[:, :NQC],
                                        bq_sb[:, m:m + 1], SCALE, add, mult)

            # K^T [(h d), k] — bkv_K is softmax-invariant, dropped
            kT_sb = const.tile([P, 2, nk], BF)
            for m in range(2):
                for nh in range((nk + 511) // 512):
                    nn_ = min(512, nk - nh * 512)
                    ps = proj_ps.tile([P, 512], FP, tag="proj")
                    for c in range(2):
                        nc.tensor.matmul(ps[:, :nn_],
                                         lhsT=wkv_sb[:, c, m * P:(m + 1) * P],
                                         rhs=xkvT_sb[:, c, nh * 512:nh * 512 + nn_],
                                         start=(c == 0), stop=(c == 1))
                    if (m * 2 + nh) % 2 == 0:
                        nc.scalar.activation(
                            kT_sb[:, m, nh * 512:nh * 512 + nn_], ps[:, :nn_],
                            mybir.ActivationFunctionType.Copy)
                    else:
                        nc.vector.tensor_copy(
                            kT_sb[:, m, nh * 512:nh * 512 + nn_], ps[:, :nn_])

            # qT/kT reshuffled so every head's d-dim sits on partitions
            # 0-31: the S^T matmuls then run at default tile position and
            # can share psum banks at different column offsets (the same
            # pattern the z matmuls use) — tile-positioned matmuls sharing
            # a bank wedge the device.  One SBUF->SBUF DMA per head.
            qT32 = const.tile([32, H, NQC], BF)
            kT32 = const.tile([32, H, nk], BF)
            for hq in range(4):
                nc.gpsimd.dma_start(qT32[:, hq::4, :],
                                    qT_sb[hq * 32:(hq + 1) * 32, :, :])
            for m in range(2):
                for hq in range(4):
                    nc.scalar.dma_start(kT32[:, m * 4 + hq, :],
                                        kT_sb[hq * 32:(hq + 1) * 32, m, :])

            # S^T[k, kc, h, q]: per (kc, head) a [k=128, q=128] matmul;
            # 4 heads batch into one psum tile with a single copy out.
            sT_sb = const.tile([P, kc_n, H, NQC], BF)

            def st_build(kc):
                for hg in range(2):
                    ps = stv_ps.tile([P, 512], FP, tag="stv", name="st_ps")
                    for hl in range(4):
                        h = hg * 4 + hl
                        nc.tensor.matmul(ps[:, hl * P:(hl + 1) * P],
                                         lhsT=kT32[:, h, kc * P:(kc + 1) * P],
                                         rhs=qT32[:, h, :],
                                         start=True, stop=True)
                    nc.vector.tensor_copy(
                        sT_sb[:, kc, hg * 4:(hg + 1) * 4, :],
                        ps.rearrange("p (hl q) -> p hl q", hl=4))

            # V [k, (h d)] (bias folded into bp on host) into vaug
            def v_build(kc):
                ps = stv_ps.tile([P, 512], FP, tag="stv", name="v_ps")
                for c in range(2):
                    nc.tensor.matmul(ps[:, :H * D],
                                     lhsT=xkvT_sb[:, c, kc * P:(kc + 1) * P],
                                     rhs=wkv_sb[:, c, H * D:2 * H * D],
                                     start=(c == 0), stop=(c == 1))
                nc.vector.tensor_copy(
                    vaug_sb[:, kc, :, 0:D],
                    ps[:, :H * D].rearrange("p (h d) -> p h d", h=H))

            st_build(0)
            st_build(1)
            v_build(0)

            # ---- main loop over k-chunks, software-pipelined: PV for chunk
            # kc is emitted after the z matmuls of chunk kc+1, so by the
            # time the PE reaches it the exp has long finished (no PE stall
            # on the cross-engine add->exp chain).
            o_ps = o_psp.tile([P, H * (D + 1)], FP)   # [q, h*(D+1)]
            xTs = {}

            def pv_emit(kc):
                xT_sb = xTs.pop(kc)
                for h in range(H):
                    # o_ps lives in one bank: open the accumulation group
                    # on the first matmul only, close on the last.
                    nc.tensor.matmul(
                        o_ps[:, h * (D + 1):(h + 1) * (D + 1)],
                        lhsT=xT_sb[:, h, :], rhs=vaug_sb[:, kc, h, :],
                        start=(kc == 0 and h == 0),
                        stop=(kc == kc_n - 1 and h == H - 1))

            for kc in range(kc_n):
                # e^T holds the chunk's logits [k, h, q] (bf16)
                e_sb = epool.tile([P, H, NQC], BF, tag="e")
                bps = []
                for hf in range(2):
                    # bias^T: one [k, 8] matmul per query into b_ps
                    # [k, (q64 h8)]; 64 queries per psum bank.
                    b_ps = b_psp.tile([P, 64 * H], FP, tag="b")
                    bps.append(b_ps)
                    for qb in (2 * hf, 2 * hf + 1):
                        z_sb = z_fetch(kc * gpc + qb)
                        for t in range(TQ):
                            ql = (qb % 2) * TQ + t
                            nc.tensor.matmul(b_ps[:, ql * H:(ql + 1) * H],
                                             lhsT=z_sb[:, t, :], rhs=wb_sb,
                                             start=(ql == 0),
                                             stop=(ql == 63))
                # adds first (critical chain), then PV two chunks back (its
                # exp is long done), then the V / S^T prefetch builds.
                for hf in range(2):
                    bv = bps[hf].rearrange("p (q h) -> p h q", h=H)
                    qs = slice(hf * 64, (hf + 1) * 64)
                    for hg in range(2):
                        hh = slice(hg * 4, (hg + 1) * 4)
                        nc.vector.tensor_tensor(
                            e_sb[:, hh, qs], sT_sb[:, kc, hh, qs],
                            bv[:, hh, :], add)
                if kc - 2 in xTs:
                    pv_emit(kc - 2)
                if kc == kc_n - 1:
                    # drain the PV backlog: PV(kc-1)'s exp completed during
                    # this chunk's z matmuls, so it is stall-free here
                    pv_emit(kc - 1)
                # exps BEFORE the v/st builds: their psum copies queue
                # behind the exps on ACT, and the next chunk's v/st matmuls
                # wait on those copies through the stv ring — emitting exps
                # first lets the copies drain without stalling the PE.
                xT_sb = xpool.tile([P, H, NQC], BF, tag="x")
                xTs[kc] = xT_sb
                for hg in range(2):          # head groups of 4
                    hh = slice(hg * 4, (hg + 1) * 4)
                    nc.scalar.activation(xT_sb[:, hh, :], e_sb[:, hh, :],
                                         mybir.ActivationFunctionType.Exp)
                if kc + 1 < kc_n:
                    v_build(kc + 1)
                if kc + 2 < kc_n:
                    st_build(kc + 2)
            pv_emit(kc_n - 1)

            # ---- epilogue: normalize, transpose, output projection ----
            recip_sb = const.tile([P, H], FP)
            nc.vector.reciprocal(
                recip_sb, o_ps.rearrange("p (h e) -> p h e", h=H)[:, :, D])
            o_sb = const.tile([P, 2, P], FP)     # [q, half, (h d)%128]
            ov = o_ps.rearrange("p (h e) -> p h e", h=H)
            rv = bass.AP(tensor=recip_sb.tensor, offset=recip_sb.offset,
                         ap=[list(recip_sb.ap[0])]
                         + [[recip_sb.ap[1][0], 4], [0, D]])
            for half in range(2):
                nc.vector.tensor_tensor(
                    o_sb[:, half, :].rearrange("p (h d) -> p h d", h=4),
                    ov[:, half * 4:(half + 1) * 4, 0:D],
                    bass.AP(tensor=rv.tensor,
                            offset=rv.offset + half * 4 * recip_sb.ap[1][0],
                            ap=rv.ap), mult)
            oT_sb = const.tile([P, 2, P], FP)
            for m in range(2):
                t_full = proj_ps.tile([P, 512], FP, tag="proj", name="t_full")
                t_ps = t_full[:, :P]
                nc.tensor.transpose(t_ps, o_sb[:, m, :], ident)
                nc.vector.tensor_copy(oT_sb[:, m, :], t_ps)
            ps = proj_ps.tile([P, 512], FP, tag="proj")
            for m in range(2):
                nc.tensor.matmul(ps[:, :CQ], lhsT=oT_sb[:, m, :],
                                 rhs=wp_sb[:, m, :], start=(m == 0), stop=False)
            nc.tensor.matmul(ps[:, :CQ], lhsT=ones_row, rhs=bp_sb,
                             start=False, stop=True)
            y_sb = const.tile([P, CQ], FP)
            nc.vector.tensor_copy(y_sb, ps[:, :CQ])
            nc.sync.dma_start(y[:], y_sb)

    nc.compile()
    return nc


def prep_inputs(x_q, x_kv, z, Wq, bq, Wkv, bkv, Wb, bb, Wp, bp, nk=1024):
    """Host-side shard prep.  Returns in_maps for the 8 cores."""
    kc_n = nk // P
    gpc = NQC // TQ
    xkvT = np.ascontiguousarray(x_kv[0].T).astype(NP_BF)     # [CKV, nk]
    # bkv_V folds into bp exactly: softmax rows sum to 1, so the +bkvV on V
    # adds bkvV @ Wp to every output row.  bb / bkv_K cancel in softmax.
    bp_eff = (np.asarray(bp, dtype=np.float64)
              + np.asarray(bkv[H * D:], dtype=np.float64)
              @ np.asarray(Wp, dtype=np.float64)).astype(np.float32)
    shared = dict(xkvT=xkvT,
                  Wq=np.ascontiguousarray(Wq).astype(NP_BF),
                  bq=np.ascontiguousarray(bq, dtype=np.float32),
                  Wkv=np.ascontiguousarray(Wkv).astype(NP_BF),
                  Wb=np.ascontiguousarray(Wb).astype(NP_BF),
                  Wp=np.ascontiguousarray(Wp, dtype=np.float32),
                  bp=bp_eff)
    in_maps = []
    for i in range(NCORES):
        qs = i * NQC
        zi = z[0, qs:qs + NQC]                           # [q, k, c]
        # -> [g=(kc,qb), c, tq, k] with q = qb*TQ + tq, k = kc*128 + k
        zi = (zi.reshape(gpc, TQ, kc_n, P, BD)
              .transpose(2, 0, 4, 1, 3)                  # [kc, qb, c, tq, k]
              .reshape(kc_n * gpc, BD, TQ, P))
        in_maps.append(dict(
            zT=np.ascontiguousarray(zi).astype(NP_F8),
            xqT=np.ascontiguousarray(x_q[0, qs:qs + NQC].T).astype(NP_BF),
            **shared,
        ))
    return in_maps


_NC_CACHE = {}


def kernel(x_q, x_kv, z, Wq, bq, Wkv, bkv, Wb, bb, Wp, bp):
    key = "full"
    if key not in _NC_CACHE:
        _NC_CACHE[key] = build_program()
    nc = _NC_CACHE[key]
    in_maps = prep_inputs(x_q, x_kv, z, Wq, bq, Wkv, bkv, Wb, bb, Wp, bp)
    res = run_bass_kernel_spmd(nc, in_maps, list(range(NCORES)))
    out = np.empty((1, NQ, CQ), dtype=np.float32)
    for i in range(NCORES):
        out[0, i * NQC:(i + 1) * NQC, :] = res.results[i]["y"]
    return out
